# revision 1
# baseline (speedup 1.0000x reference)
"""Trainium2 Bass kernel for the ContinuousVariableQNN problem.

Math reduction (validated against the jax reference on host):
  The reference builds a 256x256 symplectic matrix S from params, then
    mu   = mu0 @ S.T   with mu0[:, 0::2] = 2*inputs (odd cols zero)
    n    = (dsum + mu_x^2 + mu_p^2) / (2*hbar) - 0.5
  Because mu0's p-quadrature entries are all zero, the big matmul collapses to
    mu_dev = inputs @ Ms          with Ms[i, j] = S[j, 2*i]   ([128, 256])
  (factor 2 from displacement and the 1/4 normalization cancel), and
    n[b, m] = mu_dev[b, 2m]^2 + mu_dev[b, 2m+1]^2 + bias[m]
  with bias[m] = (diag(S S^T)[2m] + diag(S S^T)[2m+1])/4 - 0.5 (a constant).

Device strategy (pure data parallelism over 8 cores, batch-sharded):
  Per core: 16384 rows. For each 128-row tile:
    PE transpose X tile -> PSUM, DVE copy -> SBUF,
    PE matmul (fp32r)  XT.T @ Ms -> PSUM mu [128, 256],
    ACT Square -> SBUF, DVE pair-add (stride-2), GPSIMD add bias, DMA out.
  DMA layout puts CH consecutive batch rows on one partition so HBM
  transfers use multi-KB descriptors. Input DMAs ride the SP HWDGE queue,
  output DMAs the ACT HWDGE queue.
"""

import ml_dtypes
import numpy as np

import concourse.bass as bass
import concourse.mybir as mybir
import concourse.tile as tile
from concourse import bacc
from concourse.bass_utils import run_bass_kernel_spmd
from concourse.masks import make_identity

N_QUMODES = 128
N_LAYERS = 8
BATCH = 131072
N_CORES = 8
ROWS = BATCH // N_CORES          # 16384 rows per core
CH = 16                          # batch rows per partition per DMA chunk
CHUNK_ROWS = 128 * CH            # 2048
N_CHUNKS = ROWS // CHUNK_ROWS    # 8
SUBS_PER_CHUNK = CH // 4         # 4
N_SUBS = N_CHUNKS * SUBS_PER_CHUNK
SUB = 4                          # tiles (of 128 rows) per compute sub-chunk
F32 = mybir.dt.float32
F32R = mybir.dt.float32r
BF16 = mybir.dt.bfloat16


def host_prep(params: np.ndarray):
    """Build Ms [128, 256] and bias_rep [128, 512] on host (tiny, replicated)."""
    L, N = N_LAYERS, N_QUMODES
    p = params.reshape(L, N, 3).astype(np.float32)
    th1, r, th2 = p[..., 0], p[..., 1], p[..., 2]

    def rot(th):
        c, s = np.cos(th), np.sin(th)
        return np.stack([np.stack([c, -s], -1), np.stack([s, c], -1)], -2)

    z = np.zeros_like(r)
    sq = np.stack([np.stack([np.exp(-r), z], -1),
                   np.stack([z, np.exp(r)], -1)], -2)
    blk = np.einsum('lnab,lnbc,lncd->lnad', rot(th2), sq, rot(th1)).astype(np.float32)

    t = np.float32(np.cos(np.pi / 4))
    rr = np.float32(np.sin(np.pi / 4))
    BS4 = np.array([[t, 0., -rr, 0.],
                    [0., t, 0., -rr],
                    [rr, 0., t, 0.],
                    [0., rr, 0., t]], dtype=np.float32)
    C = np.eye(2 * N, dtype=np.float32)
    for i in range(N - 1):
        C[2 * i:2 * i + 4, :] = BS4 @ C[2 * i:2 * i + 4, :]

    S = np.eye(2 * N, dtype=np.float32)
    idx = np.arange(N)
    for l in range(L):
        D = np.zeros((N, 2, N, 2), np.float32)
        D[idx, :, idx, :] = blk[l]
        S = C @ (D.reshape(2 * N, 2 * N) @ S)

    # Natural interleaved column order: mu[b, 2m] = x_m, mu[b, 2m+1] = p_m.
    Ms = np.ascontiguousarray(S[:, 0::2].T, dtype=np.float32)      # [128, 256]

    dV = (S ** 2).sum(axis=1)                                      # [256]
    bias = ((dV[0::2] + dV[1::2]) / 4.0 - 0.5).astype(np.float32)  # [128]
    bias_rep = np.ascontiguousarray(
        np.tile(bias, (128, SUB)).astype(ml_dtypes.bfloat16))      # [128, 512]
    ident = np.eye(128, dtype=np.float32)
    return Ms, bias_rep, ident


def build_bass():
    nc = bacc.Bacc("TRN2", target_bir_lowering=False, debug=False,
                   num_devices=N_CORES)

    x_d = nc.dram_tensor("x", [ROWS, 128], F32R, kind="ExternalInput")
    ms_d = nc.dram_tensor("ms", [128, 256], F32R, kind="ExternalInput")
    bias_d = nc.dram_tensor("bias_rep", [128, SUB * 128], BF16,
                            kind="ExternalInput")
    ident_d = nc.dram_tensor("ident", [128, 128], F32R, kind="ExternalInput")
    out_d = nc.dram_tensor("out", [ROWS, 128], F32, kind="ExternalOutput")

    x_v = x_d.ap().rearrange("(c p r) i -> c p r i", p=128, r=CH)
    out_v = out_d.ap().rearrange("(c p r) m -> c p r m", p=128, r=CH)

    with tile.TileContext(nc) as tc:
        with (
            tc.tile_pool(name="const", bufs=1) as const_pool,
            tc.tile_pool(name="xin", bufs=3) as xin_pool,
            tc.tile_pool(name="oout", bufs=3) as oout_pool,
            tc.tile_pool(name="xts", bufs=4) as xts_pool,
            tc.tile_pool(name="sq", bufs=4) as sq_pool,
            tc.tile_pool(name="tmp", bufs=4) as tmp_pool,
            tc.tile_pool(name="xtp", bufs=2, space="PSUM") as xtp_pool,
            tc.tile_pool(name="mup", bufs=3, space="PSUM") as mup_pool,
        ):
            ident = const_pool.tile([128, 128], F32R)
            nc.sync.dma_start(out=ident, in_=ident_d.ap())

            # First input chunk next on the queue, then the remaining consts.
            x_tiles: dict[int, bass.AP] = {}
            out_tiles: dict[int, bass.AP] = {}
            xt_tiles: dict[int, bass.AP] = {}
            mu_tiles: dict[int, bass.AP] = {}
            sq_tiles: dict[int, bass.AP] = {}

            def load_chunk(c):
                x_sb = xin_pool.tile([128, CH, 128], F32R, tag="x_sb",
                                     name=f"x_sb_{c}")
                if c == 0:
                    # halve the first transfer so the PE can start sooner
                    nc.sync.dma_start(out=x_sb[:, 0:CH // 2, :],
                                      in_=x_v[c][:, 0:CH // 2, :])
                    nc.sync.dma_start(out=x_sb[:, CH // 2:, :],
                                      in_=x_v[c][:, CH // 2:, :])
                else:
                    nc.sync.dma_start(out=x_sb, in_=x_v[c])
                x_tiles[c] = x_sb
                out_tiles[c] = oout_pool.tile([128, CH, 128], F32, tag="o_sb",
                                              name=f"o_sb_{c}")

            load_chunk(0)
            ms_sb = const_pool.tile([128, 256], F32R)
            nc.sync.dma_start(out=ms_sb, in_=ms_d.ap())
            bias_sb = const_pool.tile([128, SUB * 128], BF16)
            nc.sync.dma_start(out=bias_sb, in_=bias_d.ap())

            # Software-pipelined over sub-chunks: transposes run one stage
            # ahead of the matmuls and two ahead of the elementwise tail so
            # the PE's in-order queue never waits on the DVE copy.
            for i in range(N_SUBS + 4):
                # stage A: transposes + PSUM->SBUF copy for sub-chunk i
                if i < N_SUBS:
                    c, sc = divmod(i, SUBS_PER_CHUNK)
                    if sc == 0 and c + 1 < N_CHUNKS:
                        load_chunk(c + 1)
                    x_sb = x_tiles[c]
                    xt_ps = xtp_pool.tile([128, SUB, 128], F32R)     # 1 bank
                    for q in range(SUB):
                        nc.tensor.transpose(xt_ps[:, q, :],
                                            x_sb[:, SUB * sc + q, :], ident)
                    xt_sb = xts_pool.tile([128, SUB, 128], F32R)
                    # Alternate the PSUM->SBUF copy between DVE and ACT to
                    # keep both below the DMA pace.
                    if i % 2 == 0:
                        nc.vector.tensor_copy(xt_sb, xt_ps)
                    else:
                        nc.scalar.copy(xt_sb, xt_ps)
                    xt_tiles[i] = xt_sb

                # stage B: matmuls + square for sub-chunk i-2
                t = i - 2
                if 0 <= t < N_SUBS:
                    xt_sb = xt_tiles.pop(t)
                    mu_ps = mup_pool.tile([128, SUB, 256], F32)      # 2 banks
                    for q in range(SUB):
                        nc.tensor.matmul(mu_ps[:, q, :],
                                         xt_sb[:, q, :], ms_sb,
                                         start=True, stop=True)
                    sq_sb = sq_pool.tile([128, SUB, 256], BF16)
                    # De-interleaving AP pair: reads walk mu x/p interleaved
                    # (stride 2), writes land [x-half | p-half] so the
                    # pair-add reads contiguous halves.
                    mu_v = mu_ps.rearrange("p a b -> p (a b)").rearrange(
                        "p (q m e) -> p q e m", q=SUB, e=2)
                    sq_v = sq_sb.rearrange("p a b -> p (a b)").rearrange(
                        "p (e q m) -> p q e m", e=2, q=SUB)
                    nc.scalar.activation(sq_v, mu_v,
                                         mybir.ActivationFunctionType.Square)
                    mu_tiles[t] = mu_ps
                    sq_tiles[t] = sq_sb

                # stage C: pair-add + bias + output DMA for sub-chunk i-4
                u = i - 4
                if u >= 0:
                    cu, scu = divmod(u, SUBS_PER_CHUNK)
                    mu_tiles.pop(u, None)
                    sq_sb = sq_tiles.pop(u)
                    sq_flat = sq_sb.rearrange("p a b -> p (a b)")
                    tmp_sb = tmp_pool.tile([128, SUB, 128], BF16)
                    tmp_flat = tmp_sb.rearrange("p a b -> p (a b)")
                    nc.vector.tensor_tensor(out=tmp_flat,
                                            in0=sq_flat[:, 0:SUB * 128],
                                            in1=sq_flat[:, SUB * 128:],
                                            op=mybir.AluOpType.add)
                    bias_eng = nc.gpsimd if u % 2 == 0 else nc.vector
                    bias_eng.tensor_tensor(
                        out=out_tiles[cu][:, SUB * scu:SUB * (scu + 1), :],
                        in0=tmp_sb, in1=bias_sb,
                        op=mybir.AluOpType.add)
                    if scu == SUBS_PER_CHUNK - 1:
                        nc.sync.dma_start(out=out_v[cu], in_=out_tiles.pop(cu))
                        x_tiles.pop(cu, None)

    nc.compile()
    return nc


_NC_CACHE = None


def kernel(**inputs: np.ndarray) -> np.ndarray:
    global _NC_CACHE
    X = np.ascontiguousarray(np.asarray(inputs["inputs"], dtype=np.float32))
    params = np.asarray(inputs["params"], dtype=np.float32)
    assert X.shape == (BATCH, N_QUMODES)

    Ms, bias_rep, ident = host_prep(params)

    if _NC_CACHE is None:
        _NC_CACHE = build_bass()
    nc = _NC_CACHE

    in_maps = [
        {"x": X[i * ROWS:(i + 1) * ROWS], "ms": Ms, "bias_rep": bias_rep,
         "ident": ident}
        for i in range(N_CORES)
    ]
    res = run_bass_kernel_spmd(nc, in_maps, core_ids=list(range(N_CORES)))
    out = np.concatenate([r["out"] for r in res.results], axis=0)
    return out.astype(np.float32)



# revision 7
# speedup vs baseline: 1.1587x; 1.1587x over previous
"""Trainium2 Bass kernel for the ContinuousVariableQNN problem.

Math reduction (validated against the jax reference on host):
  The reference builds a 256x256 symplectic matrix S from params, then
    mu   = mu0 @ S.T   with mu0[:, 0::2] = 2*inputs (odd cols zero)
    n    = (dsum + mu_x^2 + mu_p^2) / (2*hbar) - 0.5
  Because mu0's p-quadrature entries are all zero, the big matmul collapses to
    mu_dev = inputs @ Ms          with Ms[k, j] = S[j, 2*k]   ([128, 256])
  (factor 2 from displacement and the 1/4 normalization cancel), and
    n[b, m] = mu_dev[b, 2m]^2 + mu_dev[b, 2m+1]^2 + bias[m]
  with bias[m] = (diag(S S^T)[2m] + diag(S S^T)[2m+1])/4 - 0.5 (a constant).

Device strategy (v2, transposed/stationary-weights formulation):
  Everything runs in a TRANSPOSED layout: qumodes on partitions.
    mu_x.T = Msx.T @ X.T    (Msx = Ms[:, 0::2] stationary on the PE)
    mu_p.T = Msp.T @ X.T    (Msp = Ms[:, 1::2])
  Per core (16384 batch cols, fp16 in / bf16 out => 8.4 MB HBM traffic,
  ~23.4 us roofline @358 GB/s):
   - X.T [128, 16384] fp16 streamed in on the SP HWDGE ring (host transposes
     + casts; fp16 keeps matmul rel-err ~5e-4 vs the 2e-2 gate).
   - PE: per 1024-col group, 2 matmuls with Msx -> psum_x [128,1024] and 2
     with Msp -> psum_p (stationary weights alternate xx|pp / pp|xx so only
     one LDWEIGHTS switch per group; fp16 gets FWL automatically).
   - Tail per group: ACT Square(psum_x)->bf16, DVE psum_p*psum_p->bf16,
     then one fused scalar_tensor_tensor (sqx + bias[p]) + sqp -> out bf16
     (bias rides the per-partition scalar operand; split DVE/GPSIMD).
   - out.T [128, 16384] bf16 DMA'd out in chunks on the ACT HWDGE ring.
  Host unshards: concat cores along batch, transpose back, upcast fp32.
"""

import ml_dtypes
import numpy as np

import concourse.bass as bass
import concourse.mybir as mybir
import concourse.tile as tile
from concourse import bacc
from concourse.bass_utils import run_bass_kernel_spmd

N_QUMODES = 128
N_LAYERS = 8
BATCH = 131072
N_CORES = 8
ROWS = BATCH // N_CORES          # 16384 batch samples per core (free dim)
GROUP = 1024                     # batch cols per PSUM group (2 banks x/p each)
N_GROUPS = ROWS // GROUP         # 16
MM = 512                         # cols per matmul (1 PSUM bank)
# progressive input chunking: small first chunks let the PE start early,
# big later chunks keep DMA descriptor efficiency high
IN_CHUNKS = [1024, 1024, 2048, 4096, 8192]
OUT_CHUNK = 2048                 # output DMA granularity (4 KB/partition)
# groups whose psum_p is evacuated by an ACT Square (rest: DVE scaled copy)
SQP_ACT_GROUPS = frozenset([2, 6, 10, 14])
# psum_p evacuation scale: mu_p/16 fits fp16; compensated by x256 in the square
EVAC_SCALE = 1.0 / 16.0
EVAC_COMP = 1.0 / (EVAC_SCALE * EVAC_SCALE)
# cost-model ns per 1024-col op, for the greedy DVE/GPSIMD balance
COST_DVE_CHEAP = 326.0
COST_GPSIMD_CHEAP = 1800.0

F32 = mybir.dt.float32
F16 = mybir.dt.float16
BF16 = mybir.dt.bfloat16


def host_prep(params: np.ndarray):
    """Build Msx|Msp [128, 256] fp16 and bias [128, 1] fp32 on host.

    Returns (msxp_f16, bias_f32, scale) where inputs must be multiplied by
    `scale` before the fp16 cast (power of 2; keeps Ms inside fp16 range).
    """
    L, N = N_LAYERS, N_QUMODES
    p = params.reshape(L, N, 3).astype(np.float64)
    th1, r, th2 = p[..., 0], p[..., 1], p[..., 2]

    def rot(th):
        c, s = np.cos(th), np.sin(th)
        return np.stack([np.stack([c, -s], -1), np.stack([s, c], -1)], -2)

    z = np.zeros_like(r)
    sq = np.stack([np.stack([np.exp(-r), z], -1),
                   np.stack([z, np.exp(r)], -1)], -2)
    blk = np.einsum('lnab,lnbc,lncd->lnad', rot(th2), sq, rot(th1))

    t = np.cos(np.pi / 4)
    rr = np.sin(np.pi / 4)
    BS4 = np.array([[t, 0., -rr, 0.],
                    [0., t, 0., -rr],
                    [rr, 0., t, 0.],
                    [0., rr, 0., t]], dtype=np.float64)
    C = np.eye(2 * N)
    for i in range(N - 1):
        C[2 * i:2 * i + 4, :] = BS4 @ C[2 * i:2 * i + 4, :]

    S = np.eye(2 * N)
    idx = np.arange(N)
    for l in range(L):
        D = np.zeros((N, 2, N, 2))
        D[idx, :, idx, :] = blk[l]
        S = C @ (D.reshape(2 * N, 2 * N) @ S)

    Ms = S[:, 0::2].T                      # [128 qumodes, 256 (x|p interleaved)]
    Msx = Ms[:, 0::2]                      # [128, 128] mode m = free col m
    Msp = Ms[:, 1::2]

    # fp16 range guard: scale Ms down / X up by the same power of 2
    mmax = max(np.abs(Msx).max(), np.abs(Msp).max())
    k = max(0, int(np.ceil(np.log2(mmax / 30000.0)))) if mmax > 30000.0 else 0
    scale = float(2.0 ** k)
    msxp = np.concatenate([Msx, Msp], axis=1) / scale      # [128, 256]
    msxp_f16 = np.ascontiguousarray(msxp, dtype=np.float16)

    dV = (S ** 2).sum(axis=1)
    bias = ((dV[0::2] + dV[1::2]) / 4.0 - 0.5)
    bias_f32 = np.ascontiguousarray(bias.reshape(N, 1), dtype=np.float32)
    return msxp_f16, bias_f32, scale


def build_bass():
    nc = bacc.Bacc("TRN2", target_bir_lowering=False, debug=False,
                   num_devices=N_CORES)

    xt_d = nc.dram_tensor("xt", [128, ROWS], F16, kind="ExternalInput")
    ms_d = nc.dram_tensor("msxp", [128, 256], F16, kind="ExternalInput")
    out_d = nc.dram_tensor("out", [128, ROWS], BF16, kind="ExternalOutput")

    with tile.TileContext(nc) as tc:
        with (
            tc.tile_pool(name="const", bufs=1) as const_pool,
            tc.tile_pool(name="xin", bufs=1) as xin_pool,
            tc.tile_pool(name="oout", bufs=1) as oout_pool,
            tc.tile_pool(name="sqx", bufs=3) as sqx_pool,
            tc.tile_pool(name="sqp", bufs=3) as sqp_pool,
            tc.tile_pool(name="tp", bufs=3) as tp_pool,
            tc.tile_pool(name="psx", bufs=2, space="PSUM") as psx_pool,
            tc.tile_pool(name="psp", bufs=2, space="PSUM") as psp_pool,
        ):
            ms_sb = const_pool.tile([128, 256], F16)
            xt_sb = xin_pool.tile([128, ROWS], F16)
            out_sb = oout_pool.tile([128, ROWS], BF16)

            # consts first (tiny), then the input stream; all on the SP ring
            nc.sync.dma_start(out=ms_sb, in_=ms_d.ap())
            off = 0
            for sz in IN_CHUNKS:
                nc.sync.dma_start(out=xt_sb[:, off:off + sz],
                                  in_=xt_d.ap()[:, off:off + sz])
                off += sz
            assert off == ROWS

            msx = ms_sb[:, 0:128]
            msp = ms_sb[:, 128:256]

            # greedy DVE/GPSIMD balance for the final-add ops (bias is added
            # on the host, so the combine is a plain TT add - Pool-legal)
            dve_t = (1191.0 + COST_DVE_CHEAP) * (N_GROUPS - len(SQP_ACT_GROUPS))
            gp_t = 0.0

            def cheap_engine():
                nonlocal dve_t, gp_t
                if dve_t + COST_DVE_CHEAP <= gp_t + COST_GPSIMD_CHEAP:
                    dve_t += COST_DVE_CHEAP
                    return nc.vector
                gp_t += COST_GPSIMD_CHEAP
                return nc.gpsimd

            for g in range(N_GROUPS):
                c0 = g * GROUP
                psx_t = psx_pool.tile([128, GROUP], F32)
                psp_t = psp_pool.tile([128, GROUP], F32)
                # alternate xx|pp / pp|xx so the stationary operand switches
                # only once per group boundary
                if g % 2 == 0:
                    order = ((psx_t, msx), (psp_t, msp))
                else:
                    order = ((psp_t, msp), (psx_t, msx))
                for ps_t, w in order:
                    for h in range(GROUP // MM):
                        nc.tensor.matmul(ps_t[:, h * MM:(h + 1) * MM], w,
                                         xt_sb[:, c0 + h * MM:c0 + (h + 1) * MM],
                                         start=True, stop=True)

                sqx_t = sqx_pool.tile([128, GROUP], BF16)
                sqp_t = sqp_pool.tile([128, GROUP], BF16)
                nc.scalar.activation(sqx_t, psx_t,
                                     mybir.ActivationFunctionType.Square)
                if g in SQP_ACT_GROUPS:
                    nc.scalar.activation(sqp_t, psp_t,
                                         mybir.ActivationFunctionType.Square)
                else:
                    # DVE may read only one PSUM operand per op, so evacuate
                    # with a scaled copy (mu_p/16 fits fp16), then square in
                    # SBUF with the x256 compensation folded in.
                    tp_t = tp_pool.tile([128, GROUP], F16)
                    nc.vector.tensor_scalar(out=tp_t, in0=psp_t,
                                            scalar1=EVAC_SCALE, scalar2=None,
                                            op0=mybir.AluOpType.mult)
                    nc.vector.scalar_tensor_tensor(
                        out=sqp_t, in0=tp_t, scalar=EVAC_COMP, in1=tp_t,
                        op0=mybir.AluOpType.mult, op1=mybir.AluOpType.mult)
                cheap_engine().tensor_tensor(
                    out=out_sb[:, c0:c0 + GROUP], in0=sqx_t, in1=sqp_t,
                    op=mybir.AluOpType.add)

                # stream finished output chunks on the ACT HWDGE ring
                done = c0 + GROUP
                if done % OUT_CHUNK == 0:
                    o0 = done - OUT_CHUNK
                    nc.scalar.dma_start(out=out_d.ap()[:, o0:done],
                                        in_=out_sb[:, o0:done])

    nc.compile()
    return nc


_NC_CACHE = None


def make_in_maps(inputs: np.ndarray, params: np.ndarray):
    msxp_f16, bias_f32, scale = host_prep(params)
    X = np.asarray(inputs, dtype=np.float32)
    if scale != 1.0:
        X = X * scale
    Xh = X.astype(np.float16)
    in_maps = []
    for c in range(N_CORES):
        xt = np.ascontiguousarray(Xh[c * ROWS:(c + 1) * ROWS].T)
        in_maps.append({"xt": xt, "msxp": msxp_f16})
    return in_maps, bias_f32


def unshard(results, bias_f32) -> np.ndarray:
    bias_row = bias_f32.reshape(1, N_QUMODES)
    out = np.empty((BATCH, N_QUMODES), dtype=np.float32)
    for c, r in enumerate(results):
        out[c * ROWS:(c + 1) * ROWS] = r["out"].T.astype(np.float32) + bias_row
    return out


def kernel(**inputs: np.ndarray) -> np.ndarray:
    global _NC_CACHE
    X = inputs["inputs"]
    params = np.asarray(inputs["params"], dtype=np.float32)
    assert X.shape == (BATCH, N_QUMODES)

    in_maps, bias_f32 = make_in_maps(X, params)
    if _NC_CACHE is None:
        _NC_CACHE = build_bass()
    res = run_bass_kernel_spmd(_NC_CACHE, in_maps,
                               core_ids=list(range(N_CORES)))
    return unshard(res.results, bias_f32)
